# revision 1
# baseline (speedup 1.0000x reference)
"""Trainium2 Bass kernel for nn_AFRM_48636209660262.

Conv-BiLSTM autoencoder: 4x strided conv encoder -> channel-split BiLSTM ->
1x1 conv ffwd -> 4x conv_transpose decoder -> gamma*h + x.

Strategy: pure data parallelism over 8 NeuronCores (4 samples each, no
collectives). Activations are channel-major [C_chunk(128), B, H, W] with
zero-padded borders; convs are per-tap matmuls accumulated in PSUM
(weight-stationary loop order so each LDWEIGHTS amortizes over 2-4 psum
tiles). All 8 conv layers run in fp8e4m3 (the four big ones - enc L1/L2,
dec L2/L3 - with perf_mode=DoubleRow, K=256 per matmul; the small middle
layers as plain fp8 with per-ko weight slices since their merged-b rhs APs
exceed the TENSOR3D pattern limit); per-layer power-of-2 weight scales are
undone via the evacuation's scale AP, BN folded into weights on the host,
bias + ReLU applied by ScalarE on evacuation. conv_transpose = 4 parity
classes x 4 taps. The enc4 -> LSTM feature shuffle happens on-chip: one
shift-by-64 permutation matmul per kc plus 8 partition-aligned DVE copies
build the DoubleRow-ready seqT stationary directly (no DRAM bounce); W rows
are host-permuted to match. z_x is precomputed for all timesteps into
per-(dir,q) tiles (evacuation alternates ScalarE/VectorE so t=0 gates start
early); the recurrence folds each round's z_x slice into the h@U psum group
with a scU-scaled 4x4 identity matmul, so gates read PSUM directly with the
shared 1/scU activation scale. h^T chunks come from small PE transposes
into per-chunk H tiles so the next round's matmuls start immediately.
Residual x and the output travel as bf16; PSUM accumulation is f32
throughout. Host-side prep only reshapes/quantizes weights and the input.
"""
import numpy as np
import ml_dtypes

import concourse.mybir as mybir
import concourse.tile as tile
from concourse import bacc
from concourse.bass_utils import run_bass_kernel_spmd
from concourse.masks import make_identity

AF = mybir.ActivationFunctionType
DR = mybir.MatmulPerfMode.DoubleRow
BF16 = mybir.dt.bfloat16
F32 = mybir.dt.float32
F8 = mybir.dt.float8e4
NPF8 = ml_dtypes.float8_e4m3fn
NPBF = ml_dtypes.bfloat16

N_CORES = 8
B = 4           # batch per core
C = 256
BN_EPS = 1e-3

_CACHE: dict = {}


def _stepped(start, count, step):
    return slice(start, start + step * (count - 1) + 1, step)


def _build(gamma_nonneg=True, use_bias=False, dbg=None):
    nc = bacc.Bacc("TRN2", target_bir_lowering=False, debug=False,
                   num_devices=N_CORES)

    xin = nc.dram_tensor("xin", [128, 2, B, 66, 66], F8, kind="ExternalInput").ap()
    xres = nc.dram_tensor("xres", [2, 128, B, 64, 64], BF16, kind="ExternalInput").ap()
    # fp8 DoubleRow weights for all 8 conv layers
    # [encL1, encL2, decL2, decL3, encL3, encL4, decL0, decL1]
    w8 = nc.dram_tensor("w8", [8, 128, 16, 2, 2, 128], F8, kind="ExternalInput").ap()
    bconv = nc.dram_tensor("bconv", [128, 34], F32, kind="ExternalInput").ap()
    # LSTM mats: [Wf, Wr, Uf, Ur] x [kc, row, 4096]
    wl = nc.dram_tensor("wl", [4, 4, 128, 2, 4096], F8, kind="ExternalInput").ap()
    bl = nc.dram_tensor("bl", [2, 16, 4096], BF16, kind="ExternalInput").ap()
    wff = nc.dram_tensor("wff", [128, 4, 2, 128], F8, kind="ExternalInput").ap()
    out = nc.dram_tensor("out", [2, 128, B, 64, 64], BF16, kind="ExternalOutput").ap()

    dbg_ap = None
    dbg_shapes = {
        'l1': [2, 128, B, 34, 34], 'l2': [2, 128, B, 18, 18],
        'l3': [2, 128, B, 10, 10], 'enc4': [2, 128, B * 16],
        'seqT': [128, 4, 2, 16], 'hs': [2, 128, 8, 4, B],
        'd0': [2, 128, B, 6, 6], 'd1': [2, 128, B, 10, 10],
        'd2': [2, 128, B, 18, 18], 'd3': [2, 128, B, 34, 34],
    }
    if dbg is not None:
        dbg_ap = nc.dram_tensor("dbg", dbg_shapes[dbg], BF16,
                                kind="ExternalOutput").ap()

    with tile.TileContext(nc) as tc:
        _trace(nc, tc, xin, xres, w8, bconv, wl, bl, wff, out,
               gamma_nonneg, use_bias, dbg, dbg_ap)
    nc.compile()
    return nc


def _trace(nc, tc, xin, xres, w8, bconv, wl, bl, wff, out,
           gamma_nonneg, use_bias, dbg, dbg_ap):
    from contextlib import ExitStack

    def memset_border(t, Hp):
        nc.vector.memset(t[:, :, 0, :], 0.0)
        nc.vector.memset(t[:, :, Hp - 1, :], 0.0)
        nc.vector.memset(t[:, :, :, 0], 0.0)
        nc.vector.memset(t[:, :, :, Hp - 1], 0.0)

    # decoder parity taps: out[2m+p] <- pairs (di, k)
    ROW_TAPS = {0: [(-1, 0), (0, 2)], 1: [(0, 1), (1, 3)]}

    with ExitStack() as top:
        persist = top.enter_context(tc.tile_pool(name="persist", bufs=1))

        bias_sb = persist.tile([128, 34], F32)
        nc.sync.dma_start(bias_sb[:], bconv[:])
        ident8 = persist.tile([128, 128], BF16)
        make_identity(nc, ident8[:])
        # shift-by-64 permutation: SH[p, (p+64)%128] = 1
        shid = persist.tile([128, 128], BF16, name="shid")
        nc.vector.tensor_copy(shid[:, 64:128], ident8[:, 0:64])
        nc.vector.tensor_copy(shid[:, 0:64], ident8[:, 64:128])
        # scU-scaled 4x4 identities: fold z_x into the z_u psum group so the
        # shared 1/scU evacuation scale stays exact
        identZ = [persist.tile([4, 4], BF16, name=f"idZ{d}") for d in range(2)]
        for d in range(2):
            nc.vector.tensor_scalar_mul(identZ[d][:], ident8[0:4, 0:4],
                                        bias_sb[0:4, 32 + d:33 + d])
        H = [[[persist.tile([128, 2, 16], F8, name=f"hT{d}_{t}_{jp}")
               for jp in range(4)] for t in range(4)] for d in range(2)]
        Hm = [persist.tile([128, 8, 4, B], F8, name=f"hM{d}") for d in range(2)]
        enc4 = [persist.tile([128, 16, B], F8, name=f"enc4_{kc}")
                for kc in range(2)]  # [c, (hh,ww), b]
        d0 = persist.tile([128, 2, B, 6, 6], F8, name="d0m")
        wffsb = persist.tile([128, 4, 2, 128], F8)
        for mc in range(2):
            memset_border(d0[:, mc], 6)

        # decoder weights, all fp8 DR: [decL0, decL1, decL2, decL3]
        w8d = [persist.tile([128, 16, 2, 2, 128], F8, name=f"w8d{l}")
               for l in range(4)]

        # lwa pool spans encoder+lstm: prefetch Wf during encoder; Uf reuses
        with tc.tile_pool(name="lwa", bufs=1) as lwp:
            WLf = [lwp.tile([128, 2, 4096], F8, tag="lwa", bufs=4,
                            name=f"wf_{kp}") for kp in range(4)]

            # ================= encoder =================
            with tc.tile_pool(name="encp", bufs=1) as ep, \
                 tc.tile_pool(name="encps", bufs=1, space="PSUM") as pp:
                # fp8 DoubleRow weights [L1, L2, L3, L4] <- w8[0,1,4,5]
                w8t = [ep.tile([128, 16, 2, 2, 128], F8, tag="cw8", bufs=4,
                               name=f"w8e{l}") for l in range(4)]
                # x tiles split per (b, row-half), merged ko dim
                xt = [[ep.tile([128, 2, 34, 66], F8, tag="xcm", bufs=8,
                               name=f"x_{b}_{hf}") for hf in range(2)]
                      for b in range(B)]
                nc.sync.dma_start(xt[0][0][:], xin[:, :, 0, 0:34, :])
                nc.sync.dma_start(w8t[0][:], w8[0])
                for b in range(B):
                    for hf in range(2):
                        if b == 0 and hf == 0:
                            continue
                        nc.sync.dma_start(xt[b][hf][:],
                                          xin[:, :, b, 32 * hf:32 * hf + 34, :])
                nc.sync.dma_start(w8t[1][:], w8[1])

                l1 = ep.tile([128, 2, B, 34, 34], F8, tag="echain", bufs=4,
                             name="l1m")
                l2 = ep.tile([128, 2, B, 18, 18], F8, tag="echain", bufs=4,
                             name="l2m")
                l3 = ep.tile([128, 2, B, 10, 10], F8, tag="echain", bufs=4,
                             name="l3m")
                for ko in range(2):
                    memset_border(l1[:, ko], 34)
                    memset_border(l2[:, ko], 18)
                    memset_border(l3[:, ko], 10)

                # L1 (fp8 DR): weight-stationary, each weight streams 4 b-psums
                for hf, oh0 in ((0, 0), (1, 16)):
                    for mc in range(2):
                        pss = [pp.tile([128, 512], F32, tag="cps", bufs=6,
                                       name=f"psl1_{hf}_{mc}_{b}")
                               for b in range(B)]
                        for t in range(16):
                            kh, kw = t // 4, t % 4
                            for b in range(B):
                                rhs = xt[b][hf][:, :,
                                                _stepped(kh, 16, 2),
                                                _stepped(kw, 32, 2)]
                                nc.tensor.matmul(
                                    pss[b][:], w8t[0][:, t, mc, :, :], rhs,
                                    start=(t == 0), stop=(t == 15),
                                    perf_mode=DR)
                        for b in range(B):
                            nc.scalar.activation(
                                l1[:, mc, b, 1 + oh0:17 + oh0, 1:33], pss[b][:],
                                AF.Relu, bias=bias_sb[:, mc:mc + 1],
                                scale=bias_sb[:, 20:21])

                # deferred prefetches: issued after L1 so they don't starve
                # the critical-path x/w DMAs at kernel start
                nc.sync.dma_start(w8t[2][:], w8[4])
                nc.sync.dma_start(w8t[3][:], w8[5])
                for kp in range(4):
                    nc.sync.dma_start(WLf[kp][:], wl[0, kp])
                nc.sync.dma_start(wffsb[:], wff[:])
                for l in range(4):
                    nc.sync.dma_start(w8d[l][:], w8[[6, 7, 2, 3][l]])

                # L2 (fp8 DR): per-b groups, weight-stationary over 4 b's
                for mc in range(2):
                    pss = [pp.tile([128, 256], F32, tag="cps", bufs=6,
                                   name=f"psl2_{mc}_{b}") for b in range(B)]
                    for t in range(16):
                        kh, kw = t // 4, t % 4
                        for b in range(B):
                            rhs = l1[:, :, b,
                                     _stepped(kh, 16, 2),
                                     _stepped(kw, 16, 2)]
                            nc.tensor.matmul(
                                pss[b][:], w8t[1][:, t, mc, :, :], rhs,
                                start=(t == 0), stop=(t == 15), perf_mode=DR)
                    for b in range(B):
                        nc.scalar.activation(
                            l2[:, mc, b, 1:17, 1:17], pss[b][:],
                            AF.Relu, bias=bias_sb[:, 2 + mc:3 + mc],
                            scale=bias_sb[:, 21:22])

                # L3/L4 (fp8 non-DR, per-ko weight slices: rhs stays 3-dim)
                def enc_layer(wt, act_in, act_out, Hin, bias_idx, sc_idx):
                    OH = Hin // 2
                    for mc in range(2):
                        ps = pp.tile([128, B * OH * OH], F32, tag="cps",
                                     bufs=6, name=f"pse{Hin}_{mc}")
                        for ko in range(2):
                            for t in range(16):
                                kh, kw = t // 4, t % 4
                                rhs = act_in[:, ko, :,
                                             _stepped(kh, OH, 2),
                                             _stepped(kw, OH, 2)]
                                if act_out is None:
                                    rhs = rhs.rearrange("p b h w -> p h w b")
                                nc.tensor.matmul(
                                    ps[:], wt[:, t, mc, ko, :], rhs,
                                    start=(ko == 0 and t == 0),
                                    stop=(ko == 1 and t == 15))
                        if act_out is None:
                            nc.scalar.activation(
                                enc4[mc].rearrange("p hw b -> p (hw b)"),
                                ps[:], AF.Relu,
                                bias=bias_sb[:, bias_idx + mc:bias_idx + mc + 1],
                                scale=bias_sb[:, sc_idx:sc_idx + 1])
                        else:
                            nc.scalar.activation(
                                act_out[:, mc, :, 1:1 + OH, 1:1 + OH],
                                ps[:], AF.Relu,
                                bias=bias_sb[:, bias_idx + mc:bias_idx + mc + 1],
                                scale=bias_sb[:, sc_idx:sc_idx + 1])

                enc_layer(w8t[2], l2, l3, 16, 4, 18)
                enc_layer(w8t[3], l3, None, 8, 6, 19)

                if dbg in ('l2', 'l3'):
                    src = {'l2': l2, 'l3': l3}[dbg]
                    for mc in range(2):
                        nc.sync.dma_start(dbg_ap[mc], src[:, mc])

            if dbg == 'enc4':
                for kc in range(2):
                    nc.sync.dma_start(
                        dbg_ap[kc],
                        enc4[kc].rearrange("p hw b -> p (hw b)"))

            # ================= LSTM =================
            with tc.tile_pool(name="lstmp", bufs=1) as lp, \
                 tc.tile_pool(name="lstmps", bufs=1, space="PSUM") as lps:
                # on-chip enc4 -> seqT shuffle.  seqT row (band*64+cc) of
                # chunk (kp, ko) holds feature (hw=4kp+2band+ko, cc); cols
                # are (s,b) = (2kc+shi)*4+b.  W rows host-permuted to match.
                # Crossed half (shi != band) reads a 64-partition-swapped
                # copy made by one permutation matmul per kc.
                seqTm = lp.tile([128, 4, 2, 16], F8, name="seqTm")
                e4sw = [lps.tile([128, 64], F32, tag="pz", bufs=2,
                                 name=f"e4sw{kc}") for kc in range(2)]
                for kc in range(2):
                    nc.tensor.matmul(
                        e4sw[kc][:], shid[:],
                        enc4[kc].rearrange("p hw b -> p (hw b)"),
                        start=True, stop=True)
                for kc in range(2):
                    e4swv = e4sw[kc].rearrange("p (hw b) -> p hw b", b=B)
                    for band in range(2):
                        for am in range(2):
                            shi = band if am == 0 else 1 - band
                            s = 2 * kc + shi
                            src = (enc4[kc] if am == 0 else e4swv)
                            srcv = src[band * 64:(band + 1) * 64].rearrange(
                                "p (kp two ko) b -> p kp two ko b",
                                kp=4, two=2)[:, :, band, :, :]
                            nc.vector.tensor_copy(
                                seqTm[band * 64:(band + 1) * 64, :, :,
                                      s * 4:s * 4 + 4], srcv)
                seqT = [seqTm[:, kp] for kp in range(4)]
                if dbg == 'seqT':
                    nc.sync.dma_start(dbg_ap[:], seqTm[:])

                WLr = [lp.tile([128, 2, 4096], F8, tag="lwb", bufs=4,
                               name=f"wr_{kp}") for kp in range(4)]
                for kp in range(4):
                    nc.sync.dma_start(WLr[kp][:], wl[1, kp])

                # ---- z_x for all steps, per-(d,q) tiles so the t=0 gates
                # start as soon as their own q-slice is evacuated; evac
                # alternates scalar/vector engines.
                blt = [None, None]
                if use_bias:
                    for d in range(2):
                        blt[d] = lp.tile([16, 4096], BF16, tag="zxj", bufs=6,
                                         name=f"bl{d}")
                        nc.sync.dma_start(blt[d][:], bl[d])
                zxq = [[lp.tile([16, 1024], BF16, tag="zxj", bufs=10,
                                name=f"zx{d}_{q}") for q in range(4)]
                       for d in range(2)]
                for q in (0, 2, 3, 1):
                    for d in range(2):
                        WT = (WLf, WLr)[d]
                        wsc = bias_sb[0:16, 24 + d:25 + d]
                        ps = lps.tile([16, 1024], F32, tag="pz", bufs=2)
                        for kp in range(4):
                            for nb in range(2):
                                nc.tensor.matmul(
                                    ps[:, nb * 512:(nb + 1) * 512],
                                    seqT[kp],
                                    WT[kp][:, :, q * 1024 + nb * 512:
                                           q * 1024 + (nb + 1) * 512],
                                    start=(kp == 0), stop=(kp == 3),
                                    perf_mode=DR)
                        if use_bias:
                            nc.vector.scalar_tensor_tensor(
                                zxq[d][q][:], ps[:], wsc,
                                blt[d][:, q * 1024:(q + 1) * 1024],
                                mybir.AluOpType.mult, mybir.AluOpType.add)
                        elif (q + d) % 2 == 0:
                            nc.scalar.activation(zxq[d][q][:], ps[:],
                                                 AF.Identity, scale=wsc)
                        else:
                            nc.vector.tensor_scalar_mul(zxq[d][q][:],
                                                        ps[:], wsc)

                # U matrices reuse the W slots
                zstall = []
                for t in range(4):
                    pair = []
                    for dd in range(2):
                        zq = []
                        for q in range(4):
                            z = lp.tile([4, 1024], BF16, tag="zxj", bufs=10,
                                        name=f"zst{t}_{dd}_{q}")
                            nc.sync.dma_start(z[:],
                                              zxq[dd][q][4 * t:4 * t + 4, :])
                            zq.append(z)
                        pair.append(zq)
                    zstall.append(pair)

                ULf = [lwp.tile([128, 2, 4096], F8, tag="lwa", bufs=4,
                                name=f"uf_{kp}") for kp in range(4)]
                for kp in range(4):
                    nc.sync.dma_start(ULf[kp][:], wl[2, kp])
                ULr = [lp.tile([128, 2, 4096], F8, tag="lwb", bufs=4,
                               name=f"ur_{kp}") for kp in range(4)]
                for kp in range(4):
                    nc.sync.dma_start(ULr[kp][:], wl[3, kp])
                UL = [ULf, ULr]

                # ---- recurrence: per-dir fp8 DoubleRow z_u (M padded to 16)
                c_prev = [None, None]
                for t in range(4):
                    s_of = {0: t, 1: 3 - t}
                    zstd = zstall[t]
                    hts = []
                    for d in range(2):
                        zsums = []
                        usc = bias_sb[0:4, 26 + d:27 + d]
                        gsc = [1.0, 1.0, 1.0, 1.0]
                        if t > 0:
                            for q in range(4):
                                pz = lps.tile([16, 1024], F32, tag="pz", bufs=2)
                                for kp in range(4):
                                    for nb in range(2):
                                        nc.tensor.matmul(
                                            pz[:, nb * 512:(nb + 1) * 512],
                                            H[d][t - 1][kp][:],
                                            UL[d][kp][:, :, q * 1024 + nb * 512:
                                                      q * 1024 + (nb + 1) * 512],
                                            start=(kp == 0), stop=False,
                                            perf_mode=DR)
                                for nh in range(2):
                                    nc.tensor.matmul(
                                        pz[0:4, nh * 512:(nh + 1) * 512],
                                        identZ[d][:],
                                        zstd[d][q][:, nh * 512:(nh + 1) * 512],
                                        start=False, stop=(nh == 1),
                                        skip_group_check=True)
                                zsums.append(pz[0:4, :])
                                gsc[q] = usc
                        else:
                            zsums = [zstd[d][q][:] for q in range(4)]

                        si = lp.tile([4, 1024], BF16, tag="ltmp", bufs=8,
                                     name=f"si{t}{d}")
                        nc.scalar.activation(si[:], zsums[0][:], AF.Sigmoid,
                                             scale=gsc[0])
                        sg = lp.tile([4, 1024], BF16, tag="ltmp", bufs=8,
                                     name=f"sg{t}{d}")
                        nc.scalar.activation(sg[:], zsums[2][:], AF.Tanh,
                                             scale=gsc[2])
                        so = lp.tile([4, 1024], BF16, tag="ltmp", bufs=8,
                                     name=f"so{t}{d}")
                        nc.scalar.activation(so[:], zsums[3][:], AF.Sigmoid,
                                             scale=gsc[3])
                        c_new = lp.tile([4, 1024], BF16, tag="ltmp", bufs=8,
                                        name=f"c{t}{d}")
                        if t > 0:
                            sf = lp.tile([4, 1024], BF16, tag="ltmp", bufs=8,
                                         name=f"sf{t}{d}")
                            nc.scalar.activation(sf[:], zsums[1][:], AF.Sigmoid,
                                                 scale=gsc[1])
                            t1 = lp.tile([4, 1024], BF16, tag="ltmp", bufs=8,
                                         name=f"t1_{t}{d}")
                            nc.vector.tensor_mul(t1[:], si[:], sg[:])
                            t2 = lp.tile([4, 1024], BF16, tag="ltmp", bufs=8,
                                         name=f"t2_{t}{d}")
                            nc.vector.tensor_mul(t2[:], sf[:], c_prev[d][:])
                            nc.vector.tensor_add(c_new[:], t1[:], t2[:])
                        else:
                            nc.vector.tensor_mul(c_new[:], si[:], sg[:])
                        c_prev[d] = c_new
                        tch = lp.tile([4, 1024], BF16, tag="ltmp", bufs=8,
                                      name=f"tc{t}{d}")
                        nc.scalar.activation(tch[:], c_new[:], AF.Tanh)
                        ht = lp.tile([4, 1024], BF16, tag="lh", bufs=4,
                                     name=f"h{t}{d}")
                        nc.vector.tensor_mul(ht[:], so[:], tch[:])
                        hts.append(ht)

                    for d in range(2):
                        for j in range(8):
                            jp, ko = j // 2, j % 2
                            tp = lps.tile([128, B], BF16, tag="ptr", bufs=2)
                            nc.tensor.transpose(
                                tp[:], hts[d][:, j * 128:(j + 1) * 128],
                                ident8[0:4, 0:4])
                            if j % 2 == 0:
                                nc.scalar.copy(H[d][t][jp][:, ko, 0:4], tp[:])
                            else:
                                nc.vector.tensor_copy(H[d][t][jp][:, ko, 0:4], tp[:])

                for d in range(2):
                    for sv in range(4):
                        t = sv if d == 0 else 3 - sv
                        for j in range(8):
                            nc.vector.tensor_copy(
                                Hm[d][:, j, sv, :],
                                H[d][t][j // 2][:, j % 2, 0:4])
                if dbg == 'hs':
                    for d in range(2):
                        nc.sync.dma_start(dbg_ap[d], Hm[d][:])

                # ---- ffwd 1x1 conv + leaky relu -> d0 interior
                for mc in range(2):
                    pf = lps.tile([128, 64], F32, tag="pff", bufs=2)
                    for s in range(4):
                        for kc in range(4):
                            d, chalf = kc // 2, kc % 2
                            rhs = Hm[d][:, chalf::2, s, :]  # [128, hh, b]
                            nc.tensor.matmul(pf[:, s * 16:(s + 1) * 16],
                                             wffsb[:, kc, mc, :], rhs,
                                             start=(kc == 0), stop=(kc == 3))
                    t1 = lp.tile([128, 64], F32, tag="ltmp", bufs=8, name=f"ff{mc}")
                    nc.scalar.activation(t1[:], pf[:], AF.Identity,
                                         bias=bias_sb[:, 16 + mc:17 + mc],
                                         scale=bias_sb[:, 28:29])
                    t2 = lp.tile([128, 64], F32, tag="ltmp", bufs=8, name=f"fm{mc}")
                    nc.vector.tensor_scalar_mul(t2[:], t1[:], 0.3)
                    dst = d0[:, mc, :, 1:5, 1:5].rearrange("p b h s -> p s h b")
                    t1v = t1.rearrange("p (s h b) -> p s h b", s=4, h=4)
                    t2v = t2.rearrange("p (s h b) -> p s h b", s=4, h=4)
                    nc.vector.tensor_max(dst, t1v, t2v)

        if dbg == 'd0':
            for mc in range(2):
                nc.sync.dma_start(dbg_ap[mc], d0[:, mc])

        # ================= decoder =================
        with tc.tile_pool(name="decp", bufs=1) as dp, \
             tc.tile_pool(name="decps", bufs=1, space="PSUM") as dpp:
            d1 = dp.tile([128, 2, B, 10, 10], F8, tag="dchain", bufs=4,
                         name="d1m")
            d2 = dp.tile([128, 2, B, 18, 18], F8, tag="dchain", bufs=4,
                         name="d2m")
            d3 = dp.tile([128, 2, B, 34, 34], F8, tag="dchain", bufs=4,
                         name="d3m")
            for mc in range(2):
                memset_border(d1[:, mc], 10)
                memset_border(d2[:, mc], 18)
                memset_border(d3[:, mc], 34)

            def dec_layer_dr(wt, act_in, get_dst, Hin, bias_idx, sc_idx):
                N = B * Hin * Hin
                for mc in range(2):
                    for ph in range(2):
                        for pw in range(2):
                            ps = dpp.tile([128, N], F32, tag="dps", bufs=6,
                                          name=f"psd{Hin}_{mc}{ph}{pw}")
                            taps = [(dm, kh, dn, kw, ko)
                                    for (dm, kh) in ROW_TAPS[ph]
                                    for (dn, kw) in ROW_TAPS[pw]
                                    for ko in range(2)]
                            for i, (dm, kh, dn, kw, ko) in enumerate(taps):
                                rhs = act_in[:, ko, :,
                                             1 + dm:1 + dm + Hin,
                                             1 + dn:1 + dn + Hin]
                                nc.tensor.matmul(
                                    ps[:], wt[:, kh * 4 + kw, mc, ko, :],
                                    rhs, start=(i == 0), stop=(i == 7))
                            dst = get_dst(mc, ph, pw, Hin)
                            nc.scalar.activation(
                                dst, ps[:], AF.Relu,
                                bias=bias_sb[:, bias_idx + mc:bias_idx + mc + 1],
                                scale=bias_sb[:, sc_idx:sc_idx + 1])

            dec_layer_dr(w8d[0], d0,
                         lambda mc, ph, pw, Hin: d1[:, mc, :,
                                                    _stepped(1 + ph, Hin, 2),
                                                    _stepped(1 + pw, Hin, 2)],
                         4, 8, 29)
            dec_layer_dr(w8d[1], d1,
                         lambda mc, ph, pw, Hin: d2[:, mc, :,
                                                    _stepped(1 + ph, Hin, 2),
                                                    _stepped(1 + pw, Hin, 2)],
                         8, 10, 30)

            # dec L2 (fp8 DR): per-b, weight-stationary over 4 b-psums
            Hin = 16
            for mc in range(2):
                for ph in range(2):
                    for pw in range(2):
                        pss = [dpp.tile([128, 256], F32, tag="dps", bufs=6,
                                        name=f"psd16_{mc}{ph}{pw}_{b}")
                               for b in range(B)]
                        taps = [(dm, kh, dn, kw)
                                for (dm, kh) in ROW_TAPS[ph]
                                for (dn, kw) in ROW_TAPS[pw]]
                        for i, (dm, kh, dn, kw) in enumerate(taps):
                            for b in range(B):
                                rhs = d2[:, :, b,
                                         1 + dm:1 + dm + Hin,
                                         1 + dn:1 + dn + Hin]
                                nc.tensor.matmul(
                                    pss[b][:], w8d[2][:, kh * 4 + kw, mc, :, :],
                                    rhs, start=(i == 0), stop=(i == 3),
                                    perf_mode=DR)
                        for b in range(B):
                            dst = d3[:, mc, b,
                                     _stepped(1 + ph, Hin, 2),
                                     _stepped(1 + pw, Hin, 2)]
                            nc.scalar.activation(
                                dst, pss[b][:], AF.Relu,
                                bias=bias_sb[:, 12 + mc:13 + mc],
                                scale=bias_sb[:, 22:23])

            if dbg in ('d1',):
                for mc in range(2):
                    nc.sync.dma_start(dbg_ap[mc], d1[:, mc])

            # final layer (fp8 DR) + residual, streamed per (b, mc)
            for b in range(B):
                for mc in range(2):
                    xr = dp.tile([128, 64, 64], BF16, tag="resid", bufs=4,
                                 name=f"xr{b}_{mc}")
                    nc.sync.dma_start(xr[:], xres[mc, :, b])
                    ob = dp.tile([128, 64, 64], BF16, tag="resid", bufs=4,
                                 name=f"ob{b}_{mc}")
                    for ph in range(2):
                        for pw in range(2):
                            pss = [dpp.tile([128, 512], F32, tag="dps", bufs=6,
                                            name=f"psf{b}{mc}{ph}{pw}_{mh}")
                                   for mh in range(2)]
                            taps = [(dm, kh, dn, kw)
                                    for (dm, kh) in ROW_TAPS[ph]
                                    for (dn, kw) in ROW_TAPS[pw]]
                            for i, (dm, kh, dn, kw) in enumerate(taps):
                                for mh in range(2):
                                    m0 = mh * 16
                                    rhs = d3[:, :, b,
                                             1 + dm + m0:1 + dm + m0 + 16,
                                             1 + dn:1 + dn + 32]
                                    nc.tensor.matmul(
                                        pss[mh][:], w8d[3][:, kh * 4 + kw, mc, :, :],
                                        rhs, start=(i == 0), stop=(i == 3),
                                        perf_mode=DR)
                            for mh in range(2):
                                m0 = mh * 16
                                t1 = dp.tile([128, 512], F32, tag="fin", bufs=3,
                                             name=f"f{b}{mc}{ph}{pw}{mh}")
                                nc.scalar.activation(t1[:], pss[mh][:], AF.Relu,
                                                     bias=bias_sb[:, 14 + mc:15 + mc],
                                                     scale=bias_sb[:, 23:24])
                                oslice = ob[:, _stepped(ph + 2 * m0, 16, 2),
                                            _stepped(pw, 32, 2)]
                                xslice = xr[:, _stepped(ph + 2 * m0, 16, 2),
                                            _stepped(pw, 32, 2)]
                                t1v = t1.rearrange("p (m n) -> p m n", m=16)
                                if gamma_nonneg:
                                    nc.vector.tensor_add(oslice, t1v, xslice)
                                else:
                                    nc.vector.tensor_sub(oslice, xslice, t1v)
                    nc.sync.dma_start(out[mc, :, b], ob[:])


# --------------------------------------------------------------------------
# host-side prep + entry point
# --------------------------------------------------------------------------

def _fold_bn(w, cb, g, bb, m, v):
    A = g / np.sqrt(v + BN_EPS)
    bias = (cb - m) * A + bb
    return w * A[None, None, None, :], bias


def prep_inputs(d):
    x = np.asarray(d['x'], np.float32)
    gamma = float(np.asarray(d['gamma']).reshape(-1)[0])
    g_abs, g_nonneg = abs(gamma), gamma >= 0

    def fold(pfx, l):
        g = np.asarray(d[f'{pfx}_bn_g'][l], np.float32)
        bb = np.asarray(d[f'{pfx}_bn_b'][l], np.float32)
        m = np.asarray(d[f'{pfx}_bn_m'][l], np.float32)
        v = np.asarray(d[f'{pfx}_bn_v'][l], np.float32)
        A = g / np.sqrt(v + BN_EPS)
        bias = (np.asarray(d[f'{pfx}_b'][l], np.float32) - m) * A + bb
        return np.asarray(d[f'{pfx}_w'][l], np.float32) * A[None, None, None, :], bias

    folded = {}
    for l in range(4):
        folded[('enc', l)] = fold('enc', l)
        w, bias = fold('dec', l)
        if l == 3:
            w, bias = w * g_abs, bias * g_abs
        folded[('dec', l)] = (w, bias)

    bconv = np.zeros((128, 34), np.float32)
    for l in range(4):
        bconv[:, l * 2] = folded[('enc', l)][1][:128]
        bconv[:, l * 2 + 1] = folded[('enc', l)][1][128:]
        bconv[:, 8 + l * 2] = folded[('dec', l)][1][:128]
        bconv[:, 8 + l * 2 + 1] = folded[('dec', l)][1][128:]
    bconv[:, 16] = np.asarray(d['ffwd_b'], np.float32)[:128]
    bconv[:, 17] = np.asarray(d['ffwd_b'], np.float32)[128:]

    # fp8 DoubleRow weights for all 8 conv layers
    # [encL1, encL2, decL2, decL3, encL3, encL4, decL0, decL1]
    w8 = np.zeros((8, 128, 16, 2, 2, 128), NPF8)
    W8_KEYS = (('enc', 0), ('enc', 1), ('dec', 2), ('dec', 3),
               ('enc', 2), ('enc', 3), ('dec', 0), ('dec', 1))
    W8_SC_COLS = (20, 21, 22, 23, 18, 19, 29, 30)
    for i, key in enumerate(W8_KEYS):
        w, _ = folded[key]
        std = float(np.std(w)) + 1e-30
        sc = 2.0 ** round(np.log2(0.18 / std))
        ws = w * sc                                  # [4,4,Cin,Cout]
        # [ki, tap, mc, ko, m]: Cin = ko*128 + ki ; Cout = mc*128 + m
        w8[i] = (ws.reshape(4, 4, 2, 128, 2, 128)
                 .transpose(3, 0, 1, 4, 2, 5)
                 .reshape(128, 16, 2, 2, 128).astype(NPF8))
        bconv[:, W8_SC_COLS[i]] = 1.0 / sc

    def permW(w):
        # seq feature l = pix*64 + cc -> device row
        # l' = (pix//4)*256 + (pix%2)*128 + ((pix//2)%2)*64 + cc
        w4 = np.asarray(w).reshape(16, 64, 4096)
        out = np.empty((4, 2, 2, 64, 4096), w4.dtype)
        for pix in range(16):
            out[pix // 4, pix % 2, (pix // 2) % 2] = w4[pix]
        return np.ascontiguousarray(out.reshape(1024, 4096))

    wlf32 = [permW(np.asarray(d['lstm_fwd_W'], np.float32)),
             permW(np.asarray(d['lstm_rvs_W'], np.float32)),
             np.asarray(d['lstm_fwd_U'], np.float32),
             np.asarray(d['lstm_rvs_U'], np.float32)]
    wl = np.zeros((4, 4, 128, 2, 4096), NPF8)
    for i, m in enumerate(wlf32):
        std = float(np.std(m)) + 1e-30
        sc = 2.0 ** round(np.log2(0.18 / std))
        # row r = kp*256 + ko*128 + ki
        wl[i] = (m * sc).reshape(4, 2, 128, 4096).transpose(0, 2, 1, 3) \
                        .astype(NPF8)
        if i < 2:
            bconv[0:16, 24 + i] = 1.0 / sc
        else:
            bconv[0:4, 24 + i] = 1.0 / sc   # cols 26 (Uf), 27 (Ur)
            bconv[0:4, 30 + i] = sc         # cols 32/33: identZ fold scale
    blv = np.stack([np.asarray(d['lstm_fwd_b'], np.float32),
                    np.asarray(d['lstm_rvs_b'], np.float32)])
    use_bias = bool(np.any(blv != 0))
    bl = np.broadcast_to(blv[:, None, :], (2, 16, 4096)).astype(NPBF).copy()

    wffv = np.asarray(d['ffwd_w'], np.float32)[0, 0]     # [512, 256]
    stdf = float(np.std(wffv)) + 1e-30
    scf = 2.0 ** round(np.log2(0.18 / stdf))
    bconv[:, 28] = 1.0 / scf
    wff = np.ascontiguousarray(
        (wffv * scf).reshape(4, 128, 2, 128).transpose(1, 0, 2, 3).astype(NPF8))

    xcm = np.zeros((N_CORES, 128, 2, B, 66, 66), NPF8)
    xrs = np.zeros((N_CORES, 2, 128, B, 64, 64), NPBF)
    xt = x.reshape(N_CORES, B, 64, 64, 2, 128).transpose(0, 4, 5, 1, 2, 3)
    xcm[:, :, :, :, 1:65, 1:65] = xt.transpose(0, 2, 1, 3, 4, 5).astype(NPF8)
    xrs[:] = xt.astype(NPBF)

    in_maps = []
    for c in range(N_CORES):
        in_maps.append(dict(xin=xcm[c], xres=xrs[c], w8=w8,
                            bconv=bconv, wl=wl, bl=bl, wff=wff))
    return in_maps, g_nonneg, use_bias


def get_nc(g_nonneg=True, use_bias=False, dbg=None):
    key = (g_nonneg, use_bias, dbg)
    if key not in _CACHE:
        _CACHE[key] = _build(gamma_nonneg=g_nonneg, use_bias=use_bias, dbg=dbg)
    return _CACHE[key]


def kernel(**inputs):
    in_maps, g_nonneg, use_bias = prep_inputs(inputs)
    nc = get_nc(g_nonneg, use_bias)
    res = run_bass_kernel_spmd(nc, in_maps, core_ids=list(range(N_CORES)))
    outs = []
    for c in range(N_CORES):
        o = np.asarray(res.results[c]["out"], np.float32)
        outs.append(o.transpose(2, 3, 4, 0, 1).reshape(B, 64, 64, 256))
    return np.concatenate(outs, axis=0).astype(np.float32)

